# revision 5
# baseline (speedup 1.0000x reference)
"""Trainium2 Bass kernel for the autoregressive VAE (3-layer enc/dec LSTM).

Strategy: 8-way tensor parallelism over the hidden/gate dimension.
Core k owns h-indices [128k:128k+128) of every LSTM layer (enc+dec).
Per layer+step it computes its [B=64, 512] gate slice (batch-major
matmuls: activations stationary, weights moving), runs the cell
elementwise update for its h-slice, transposes h to [128, 64] and
all-gathers h.T across the 8 cores.  Heads are replicated on every
core so the z -> decoder and logits -> next-step-encoder feedbacks
need no communication.

v2 serial-path optimizations over the original baseline:
- mu and logvar heads fused into one [B, 512] matmul group.
- encoder-input fold: gates_e0 = b + rec + x_t @ (W1+W2)^T
  + sigmoid(logits) @ (-W2)^T, so the x-dependent terms accumulate
  off-chain and only two sigT matmuls are on the critical path
  (drops the xhat subtraction and the per-step x_f load).
- all six layers' bias+recurrent matmul groups are emitted at step
  start so the PE works during gather/cell gaps (psum pool bufs=6).
- cell: tanh(g) first, f*c on gpsimd in parallel with i*g on vector.
"""

import sys

sys.path.insert(0, "/opt/trn_rl_repo")

import numpy as np
import ml_dtypes

from concourse import bass, tile, mybir, bacc
from concourse.bass_utils import run_bass_kernel_spmd

BF16 = ml_dtypes.bfloat16
L, H, D, B, T_FULL = 3, 1024, 256, 64, 128
NC = 8
SL = H // NC          # 128 h-indices per core
G = 4 * SL            # 512 gate rows per core
AF = mybir.ActivationFunctionType


def _chunked_moving(WT, n_cols):
    """[K, n_cols] -> SBUF layout [128, (K//128)*n_cols] bf16, chunk-major."""
    K = WT.shape[0]
    assert K % 128 == 0
    return (
        WT.reshape(K // 128, 128, n_cols)
        .transpose(1, 0, 2)
        .reshape(128, (K // 128) * n_cols)
        .astype(BF16)
    )


def prepare_core_inputs(inputs, core, T=T_FULL):
    """Host-side preprocessing of one core's input map."""
    i = inputs
    rows = np.concatenate(
        [np.arange(g * H + SL * core, g * H + SL * core + SL) for g in range(4)]
    )
    m = {}
    layers = [
        ("e1", i["enc_Wih"][0], i["enc_Whh"][1], i["enc_b"][1]),
        ("e2", i["enc_Wih"][1], i["enc_Whh"][2], i["enc_b"][2]),
        ("d0", i["dec_Wih0"], i["dec_Whh"][0], i["dec_b"][0]),
        ("d1", i["dec_Wih"][0], i["dec_Whh"][1], i["dec_b"][1]),
        ("d2", i["dec_Wih"][1], i["dec_Whh"][2], i["dec_b"][2]),
    ]
    for name, Wih, Whh, b in layers:
        m[f"wih_{name}"] = _chunked_moving(np.ascontiguousarray(Wih[rows].T), G)
        m[f"whh_{name}"] = _chunked_moving(np.ascontiguousarray(Whh[rows].T), G)
        m[f"b_{name}"] = b[rows].astype(BF16).reshape(1, G)
    # e0 with the x-fold: W12 = W1+W2 (for x), wn2 = -W2 (for sigmoid(lg))
    W0 = i["enc_Wih0"][rows]                      # [G, 2D]
    W12 = (W0[:, :D] + W0[:, D:]).T               # [D, G]
    WN2 = (-W0[:, D:]).T                          # [D, G]
    m["wih_e0"] = _chunked_moving(np.ascontiguousarray(W12), G)
    m["wn2_e0"] = _chunked_moving(np.ascontiguousarray(WN2), G)
    m["whh_e0"] = _chunked_moving(np.ascontiguousarray(i["enc_Whh"][0][rows].T), G)
    m["b_e0"] = i["enc_b"][0][rows].astype(BF16).reshape(1, G)
    # heads, replicated on every core; mu and logvar fused -> [2D, H]
    Wmuv = np.concatenate([i["W_mu"], i["W_logvar"]], axis=0)     # [2D, H]
    m["w_muv"] = _chunked_moving(np.ascontiguousarray(Wmuv.T), 2 * D)
    m["w_out"] = _chunked_moving(np.ascontiguousarray(i["W_out"].T), D)
    m["b_muv"] = np.concatenate([i["b_mu"], i["b_logvar"]]).astype(BF16).reshape(1, 2 * D)
    m["b_out"] = i["b_out"].astype(BF16).reshape(1, D)
    # x transposed for stationary use: [T, 128, 2*64] (partition-major)
    xT = (
        i["x"][:, :T]
        .transpose(1, 2, 0)               # [T, D, B]
        .reshape(T, 2, 128, B)
        .transpose(0, 2, 1, 3)            # [T, 128, 2, B]
        .astype(BF16)
        .reshape(T, 128, 2 * B)
    )
    m["xT"] = np.ascontiguousarray(xT)
    m["eps_f"] = np.ascontiguousarray(i["eps"][:, :T].transpose(1, 0, 2)).astype(
        np.float32
    )
    m["ones"] = np.ones((1, B), BF16)
    m["ident"] = np.eye(128, dtype=BF16)
    return m


def build(T=T_FULL, steps=None, gather_mode='cc'):
    nc = bacc.Bacc("TRN2", target_bir_lowering=False, num_devices=NC)
    f32, bf16 = mybir.dt.float32, mybir.dt.bfloat16

    lay_names = ["e0", "e1", "e2", "d0", "d1", "d2"]
    n_in_chunks = {"e0": 2, "e1": 8, "e2": 8, "d0": 2, "d1": 8, "d2": 8}

    di = {}  # dram inputs
    for n in lay_names:
        di[f"wih_{n}"] = nc.dram_tensor(
            f"wih_{n}", [128, n_in_chunks[n] * G], bf16, kind="ExternalInput"
        )
        di[f"whh_{n}"] = nc.dram_tensor(f"whh_{n}", [128, 8 * G], bf16, kind="ExternalInput")
        di[f"b_{n}"] = nc.dram_tensor(f"b_{n}", [1, G], bf16, kind="ExternalInput")
    di["wn2_e0"] = nc.dram_tensor("wn2_e0", [128, 2 * G], bf16, kind="ExternalInput")
    di["w_muv"] = nc.dram_tensor("w_muv", [128, 8 * 2 * D], bf16, kind="ExternalInput")
    di["w_out"] = nc.dram_tensor("w_out", [128, 8 * D], bf16, kind="ExternalInput")
    di["b_muv"] = nc.dram_tensor("b_muv", [1, 2 * D], bf16, kind="ExternalInput")
    di["b_out"] = nc.dram_tensor("b_out", [1, D], bf16, kind="ExternalInput")
    di["xT"] = nc.dram_tensor("xT", [T, 128, 2 * B], bf16, kind="ExternalInput")
    di["eps_f"] = nc.dram_tensor("eps_f", [T, B, D], f32, kind="ExternalInput")
    di["ones"] = nc.dram_tensor("ones", [1, B], bf16, kind="ExternalInput")
    di["ident"] = nc.dram_tensor("ident", [128, 128], bf16, kind="ExternalInput")

    out_mu = nc.dram_tensor("out_mu", [T, B, D], f32, kind="ExternalOutput")
    out_lv = nc.dram_tensor("out_lv", [T, B, D], f32, kind="ExternalOutput")
    out_z = nc.dram_tensor("out_z", [T, B, D], f32, kind="ExternalOutput")
    out_lg = nc.dram_tensor("out_lg", [T, B, D], f32, kind="ExternalOutput")

    rg = [list(range(NC))]
    n_steps = T if steps is None else steps

    with tile.TileContext(nc) as tc:
        with (
            tc.tile_pool(name="wpool", bufs=1) as wp,
            tc.tile_pool(name="state", bufs=1) as st,
            tc.tile_pool(name="xio", bufs=4) as xio,
            tc.tile_pool(name="tmp", bufs=2) as tp,
            tc.tile_pool(name="psg", bufs=1, space="PSUM") as psg,
            tc.tile_pool(name="psh", bufs=1, space="PSUM") as psh,
            tc.tile_pool(name="pst", bufs=1, space="PSUM") as pst,
            tc.tile_pool(name="dio", bufs=2, space="DRAM") as dio,
        ):
            # ---- load weights into SBUF (persistent) ----
            w = {}
            for name, dt_ in list(di.items()):
                if name in ("xT", "eps_f"):
                    continue
                shape = list(di[name].shape)
                t = wp.tile(shape, di[name].dtype, tag=f"w_{name}", name=f"w_{name}")
                nc.sync.dma_start(t[:], di[name][:])
                w[name] = t

            # ---- persistent state ----
            c_st = {}
            g_h = {}
            for n in lay_names:
                c_st[n] = st.tile([B, SL], f32, tag=f"c_{n}", name=f"c_{n}")
                nc.vector.memset(c_st[n][:], 0.0)
                g_h[n] = st.tile([128, NC * B], bf16, tag=f"gh_{n}", name=f"gh_{n}")
                nc.vector.memset(g_h[n][:], 0.0)
            # sigT holds sigmoid(logits(t-1))^T; sigmoid(0) = 0.5 at t=0
            sigT = st.tile([128, 2 * B], bf16, tag="sigT", name="sigT")
            nc.vector.memset(sigT[:], 0.5)

            ident64 = w["ident"][0:64, 0:64]

            def transpose_to(dst_ap, src_ap):
                """src [64,<=128] sbuf -> dst [<=128,64] sbuf (via PE + copy)."""
                ptile = pst.tile([128, B], bf16, tag="pt", name="pt")
                nc.tensor.transpose(ptile[:], src_ap, ident64)
                nc.vector.tensor_copy(dst_ap, ptile[:])

            def emit_bias_rec(name, psum):
                """bias + recurrent terms into psum (off critical path)."""
                whh, b = w[f"whh_{name}"], w[f"b_{name}"]
                nc.tensor.matmul(
                    psum[:, :], w["ones"][0:1, 0:B], b[0:1, :], start=True, stop=False
                )
                ghl = g_h[name]
                for cix in range(8):
                    nc.tensor.matmul(
                        psum[:, :],
                        ghl[:, 64 * cix : 64 * cix + 64],
                        whh[:, cix * G : (cix + 1) * G],
                        start=False,
                        stop=False,
                    )

            def emit_input(name, psum, in_stat, wkey=None):
                wih = w[wkey or f"wih_{name}"]
                n_in = len(in_stat)
                for cix, stat in enumerate(in_stat):
                    nc.tensor.matmul(
                        psum[:, :],
                        stat,
                        wih[:, cix * G : (cix + 1) * G],
                        start=False,
                        stop=False,
                    )

            def emit_stop(name, psum, in_stat, wkey=None):
                """last input chunk with stop=True"""
                wih = w[wkey or f"wih_{name}"]
                nc.tensor.matmul(
                    psum[:, :], in_stat[0], wih[:, in_stat[1] * G : (in_stat[1] + 1) * G],
                    start=False, stop=True,
                )

            def emit_cell(name, psum):
                """gates psum -> new h (bf16 [64,128]) ; updates c state."""
                tanh_g = tp.tile([B, 128], f32, tag="tanh_g", name="tanh_g")
                nc.scalar.activation(tanh_g[:], psum[:, 256:384], AF.Tanh)
                sig_if = tp.tile([B, 256], f32, tag="sig_if", name="sig_if")
                nc.scalar.activation(sig_if[:], psum[:, 0:256], AF.Sigmoid)
                sig_o = tp.tile([B, 128], f32, tag="sig_o", name="sig_o")
                nc.scalar.activation(sig_o[:], psum[:, 384:512], AF.Sigmoid)
                t2 = tp.tile([B, 128], f32, tag="t2", name="t2")
                nc.vector.tensor_mul(t2[:], sig_if[:, 128:256], c_st[name][:])
                t1 = tp.tile([B, 128], f32, tag="t1", name="t1")
                nc.vector.tensor_mul(t1[:], sig_if[:, 0:128], tanh_g[:])
                nc.vector.tensor_add(c_st[name][:], t1[:], t2[:])
                tanh_c = tp.tile([B, 128], f32, tag="tanh_c", name="tanh_c")
                nc.scalar.activation(tanh_c[:], c_st[name][:], AF.Tanh)
                h_bf = tp.tile([B, 128], bf16, tag="h_bf", name="h_bf")
                nc.vector.tensor_mul(h_bf[:], sig_o[:], tanh_c[:])
                return h_bf

            def emit_gather(name, h_bf):
                send = tp.tile([128, B], bf16, tag="send", name="send")
                transpose_to(send[:], h_bf[:])
                if gather_mode == "none":
                    for s in range(NC):
                        nc.vector.tensor_copy(g_h[name][:, s * B : (s + 1) * B], send[:])
                    return
                ib = dio.tile([128, B], bf16, tag="ib", name="ib")
                nc.sync.dma_start(ib[:], send[:])
                ob = dio.tile([NC, 128, B], bf16, tag="ob", name="ob")
                nc.gpsimd.collective_compute(
                    "AllGather",
                    mybir.AluOpType.bypass,
                    replica_groups=rg,
                    ins=[ib.opt()],
                    outs=[ob.opt()],
                )
                for s in range(NC):
                    nc.sync.dma_start(g_h[name][:, s * B : (s + 1) * B], ob[s])

            def emit_head(wname, bname, stat_buf, psum, ncols):
                emit_head2(wname, bname, stat_buf, psum[:, :], ncols)

            def emit_head2(wname, bname, stat_buf, psum_ap, ncols):
                nc.tensor.matmul(
                    psum_ap, w["ones"][0:1, 0:B], w[bname][0:1, :],
                    start=True, stop=False,
                )
                for cix in range(8):
                    nc.tensor.matmul(
                        psum_ap,
                        stat_buf[:, 64 * cix : 64 * cix + 64],
                        w[wname][:, cix * ncols : (cix + 1) * ncols],
                        start=False,
                        stop=(cix == 7),
                    )

            # ================= time loop =================
            for t in range(n_steps):
                xT_t = xio.tile([128, 2 * B], bf16, tag="xT", name="xT")
                nc.sync.dma_start(xT_t[:], di["xT"][t])
                eps_t = xio.tile([B, D], f32, tag="eps", name="eps")
                nc.sync.dma_start(eps_t[:], di["eps_f"][t])

                # ---- bias + recurrent for all layers, plus e0's x-terms:
                # everything that does not depend on this step's gathers.
                psums = {}
                for name in lay_names:
                    psums[name] = psg.tile([B, G], f32, tag=f"ps_{name}", name=f"ps_{name}")
                    emit_bias_rec(name, psums[name])
                emit_input("e0", psums["e0"], [xT_t[:, 0:64], xT_t[:, 64:128]])

                # ---- encoder ----
                # e0: only the sigT terms are chain-dependent
                emit_input("e0", psums["e0"], [sigT[:, 0:64]], wkey="wn2_e0")
                emit_stop("e0", psums["e0"], (sigT[:, 64:128], 1), wkey="wn2_e0")
                h_bf = emit_cell("e0", psums["e0"])
                emit_gather("e0", h_bf)

                for name, prev in (("e1", "e0"), ("e2", "e1")):
                    pg = g_h[prev]
                    emit_input(name, psums[name],
                               [pg[:, 64 * cx : 64 * cx + 64] for cx in range(7)])
                    emit_stop(name, psums[name], (pg[:, 448:512], 7))
                    h_bf = emit_cell(name, psums[name])
                    emit_gather(name, h_bf)

                # ---- fused mu/logvar head + z ----
                pm = psh.tile([B, 2 * D], f32, tag="psh_muv", name="psh_muv")
                emit_head("w_muv", "b_muv", g_h["e2"], pm, 2 * D)
                muv_sb = tp.tile([B, 2 * D], f32, tag="muv_sb", name="muv_sb")
                nc.vector.tensor_copy(muv_sb[:], pm[:, :])
                nc.sync.dma_start(out_mu[t], muv_sb[:, 0:D])
                nc.sync.dma_start(out_lv[t], muv_sb[:, D : 2 * D])

                std_t = tp.tile([B, D], f32, tag="std", name="std")
                nc.scalar.activation(std_t[:], muv_sb[:, D : 2 * D], AF.Exp, scale=0.5)
                tz = tp.tile([B, D], f32, tag="tz", name="tz")
                nc.vector.tensor_mul(tz[:], eps_t[:], std_t[:])
                z_sb = tp.tile([B, D], f32, tag="z_sb", name="z_sb")
                nc.vector.tensor_add(z_sb[:], muv_sb[:, 0:D], tz[:])
                nc.sync.dma_start(out_z[t], z_sb[:])
                z_bf = tp.tile([B, D], bf16, tag="z_bf", name="z_bf")
                nc.vector.tensor_copy(z_bf[:], z_sb[:])
                zT = tp.tile([128, 2 * B], bf16, tag="zT", name="zT")
                for cix in range(2):
                    transpose_to(
                        zT[:, cix * B : (cix + 1) * B],
                        z_bf[:, cix * 128 : (cix + 1) * 128],
                    )

                # ---- decoder ----
                emit_input("d0", psums["d0"], [zT[:, 0:64]])
                emit_stop("d0", psums["d0"], (zT[:, 64:128], 1))
                h_bf = emit_cell("d0", psums["d0"])
                emit_gather("d0", h_bf)

                for name, prev in (("d1", "d0"), ("d2", "d1")):
                    pg = g_h[prev]
                    emit_input(name, psums[name],
                               [pg[:, 64 * cx : 64 * cx + 64] for cx in range(7)])
                    emit_stop(name, psums[name], (pg[:, 448:512], 7))
                    h_bf = emit_cell(name, psums[name])
                    emit_gather(name, h_bf)

                # ---- output head ----
                po_full = psh.tile([B, 2 * D], f32, tag="psh_muv", name="psh_muv")
                po = po_full[:, 0:D]
                emit_head2("w_out", "b_out", g_h["d2"], po, D)
                lg_sb = tp.tile([B, D], f32, tag="lg_sb", name="lg_sb")
                nc.vector.tensor_copy(lg_sb[:], po)
                nc.sync.dma_start(out_lg[t], lg_sb[:])

                # ---- sigT for t+1 ----
                if t + 1 < n_steps:
                    sig_lg = tp.tile([B, D], bf16, tag="sig_lg", name="sig_lg")
                    nc.scalar.activation(sig_lg[:], po, AF.Sigmoid)
                    for cix in range(2):
                        transpose_to(
                            sigT[:, cix * B : (cix + 1) * B],
                            sig_lg[:, cix * 128 : (cix + 1) * 128],
                        )

    nc.compile()
    return nc


_CACHE = {}


def run(inputs, T=T_FULL, trace=False):
    if T not in _CACHE:
        _CACHE[T] = build(T)
    nc = _CACHE[T]
    in_maps = [prepare_core_inputs(inputs, k, T) for k in range(NC)]
    res = run_bass_kernel_spmd(nc, in_maps, core_ids=list(range(NC)), trace=trace)
    r = res.results[0]
    sig = lambda v: 1.0 / (1.0 + np.exp(-v))
    tb = lambda a: np.ascontiguousarray(np.swapaxes(a, 0, 1))
    logits = tb(r["out_lg"])
    return (
        sig(logits).astype(np.float32),
        logits,
        tb(r["out_mu"]),
        tb(r["out_lv"]),
        tb(r["out_z"]),
    )


def kernel(**inputs):
    inputs = {k: np.asarray(v) for k, v in inputs.items()}
    return run(inputs, T=T_FULL)


# revision 10
# speedup vs baseline: 1.0173x; 1.0173x over previous
"""Trainium2 Bass kernel for the autoregressive VAE (3-layer enc/dec LSTM).

Strategy: 8-way tensor parallelism over the hidden/gate dimension.
Core k owns h-indices [128k:128k+128) of every LSTM layer (enc+dec).
Per layer+step it computes its [B=64, 512] gate slice (batch-major
matmuls: activations stationary, weights moving), runs the cell
elementwise update for its h-slice, transposes h to [128, 64] and
all-gathers h.T across the 8 cores.  Heads are replicated on every
core so the z -> decoder and logits -> next-step-encoder feedbacks
need no communication.

v2 serial-path optimizations over the original baseline:
- mu and logvar heads fused into one [B, 512] matmul group.
- encoder-input fold: gates_e0 = b + rec + x_t @ (W1+W2)^T
  + sigmoid(logits) @ (-W2)^T, so the x-dependent terms accumulate
  off-chain and only two sigT matmuls are on the critical path
  (drops the xhat subtraction and the per-step x_f load).
- all six layers' bias+recurrent matmul groups are emitted at step
  start so the PE works during gather/cell gaps (psum pool bufs=6).
- cell: tanh(g) first, f*c on gpsimd in parallel with i*g on vector.
"""

import sys

sys.path.insert(0, "/opt/trn_rl_repo")

import numpy as np
import ml_dtypes

from concourse import bass, tile, mybir, bacc
from concourse.bass_utils import run_bass_kernel_spmd

BF16 = ml_dtypes.bfloat16
L, H, D, B, T_FULL = 3, 1024, 256, 64, 128
NC = 8
SL = H // NC          # 128 h-indices per core
G = 4 * SL            # 512 gate rows per core
AF = mybir.ActivationFunctionType


def _chunked_moving(WT, n_cols):
    """[K, n_cols] -> SBUF layout [128, (K//128)*n_cols] bf16, chunk-major."""
    K = WT.shape[0]
    assert K % 128 == 0
    return (
        WT.reshape(K // 128, 128, n_cols)
        .transpose(1, 0, 2)
        .reshape(128, (K // 128) * n_cols)
        .astype(BF16)
    )


def prepare_core_inputs(inputs, core, T=T_FULL):
    """Host-side preprocessing of one core's input map."""
    i = inputs
    rows = np.concatenate(
        [np.arange(g * H + SL * core, g * H + SL * core + SL) for g in range(4)]
    )
    m = {}
    layers = [
        ("e1", i["enc_Wih"][0], i["enc_Whh"][1], i["enc_b"][1]),
        ("e2", i["enc_Wih"][1], i["enc_Whh"][2], i["enc_b"][2]),
        ("d0", i["dec_Wih0"], i["dec_Whh"][0], i["dec_b"][0]),
        ("d1", i["dec_Wih"][0], i["dec_Whh"][1], i["dec_b"][1]),
        ("d2", i["dec_Wih"][1], i["dec_Whh"][2], i["dec_b"][2]),
    ]
    for name, Wih, Whh, b in layers:
        m[f"wih_{name}"] = _chunked_moving(np.ascontiguousarray(Wih[rows].T), G)
        m[f"whh_{name}"] = _chunked_moving(np.ascontiguousarray(Whh[rows].T), G)
        m[f"b_{name}"] = b[rows].astype(BF16).reshape(1, G)
    # e0 with the x-fold: W12 = W1+W2 (for x), wn2 = -W2 (for sigmoid(lg))
    W0 = i["enc_Wih0"][rows]                      # [G, 2D]
    W12 = (W0[:, :D] + W0[:, D:]).T               # [D, G]
    WN2 = (-W0[:, D:]).T                          # [D, G]
    m["wih_e0"] = _chunked_moving(np.ascontiguousarray(W12), G)
    m["wn2_e0"] = _chunked_moving(np.ascontiguousarray(WN2), G)
    m["whh_e0"] = _chunked_moving(np.ascontiguousarray(i["enc_Whh"][0][rows].T), G)
    m["b_e0"] = i["enc_b"][0][rows].astype(BF16).reshape(1, G)
    # heads, replicated on every core; mu and logvar fused -> [2D, H]
    Wmuv = np.concatenate([i["W_mu"], i["W_logvar"]], axis=0)     # [2D, H]
    m["w_muv"] = _chunked_moving(np.ascontiguousarray(Wmuv.T), 2 * D)
    m["w_out"] = _chunked_moving(np.ascontiguousarray(i["W_out"].T), D)
    m["b_muv"] = np.concatenate([i["b_mu"], i["b_logvar"]]).astype(BF16).reshape(1, 2 * D)
    m["b_out"] = i["b_out"].astype(BF16).reshape(1, D)
    # x transposed for stationary use: [T, 128, 2*64] (partition-major)
    xT = (
        i["x"][:, :T]
        .transpose(1, 2, 0)               # [T, D, B]
        .reshape(T, 2, 128, B)
        .transpose(0, 2, 1, 3)            # [T, 128, 2, B]
        .astype(BF16)
        .reshape(T, 128, 2 * B)
    )
    m["xT"] = np.ascontiguousarray(xT)
    m["eps_f"] = np.ascontiguousarray(i["eps"][:, :T].transpose(1, 0, 2)).astype(
        np.float32
    )
    m["ones"] = np.ones((1, B), BF16)
    m["ident"] = np.eye(128, dtype=BF16)
    return m


def build(T=T_FULL, steps=None, gather_mode='cc'):
    nc = bacc.Bacc("TRN2", target_bir_lowering=False, num_devices=NC)
    f32, bf16 = mybir.dt.float32, mybir.dt.bfloat16

    lay_names = ["e0", "e1", "e2", "d0", "d1", "d2"]
    n_in_chunks = {"e0": 2, "e1": 8, "e2": 8, "d0": 2, "d1": 8, "d2": 8}

    di = {}  # dram inputs
    for n in lay_names:
        di[f"wih_{n}"] = nc.dram_tensor(
            f"wih_{n}", [128, n_in_chunks[n] * G], bf16, kind="ExternalInput"
        )
        di[f"whh_{n}"] = nc.dram_tensor(f"whh_{n}", [128, 8 * G], bf16, kind="ExternalInput")
        di[f"b_{n}"] = nc.dram_tensor(f"b_{n}", [1, G], bf16, kind="ExternalInput")
    di["wn2_e0"] = nc.dram_tensor("wn2_e0", [128, 2 * G], bf16, kind="ExternalInput")
    di["w_muv"] = nc.dram_tensor("w_muv", [128, 8 * 2 * D], bf16, kind="ExternalInput")
    di["w_out"] = nc.dram_tensor("w_out", [128, 8 * D], bf16, kind="ExternalInput")
    di["b_muv"] = nc.dram_tensor("b_muv", [1, 2 * D], bf16, kind="ExternalInput")
    di["b_out"] = nc.dram_tensor("b_out", [1, D], bf16, kind="ExternalInput")
    di["xT"] = nc.dram_tensor("xT", [T, 128, 2 * B], bf16, kind="ExternalInput")
    di["eps_f"] = nc.dram_tensor("eps_f", [T, B, D], f32, kind="ExternalInput")
    di["ones"] = nc.dram_tensor("ones", [1, B], bf16, kind="ExternalInput")
    di["ident"] = nc.dram_tensor("ident", [128, 128], bf16, kind="ExternalInput")

    out_mu = nc.dram_tensor("out_mu", [T, B, D], f32, kind="ExternalOutput")
    out_lv = nc.dram_tensor("out_lv", [T, B, D], f32, kind="ExternalOutput")
    out_z = nc.dram_tensor("out_z", [T, B, D], f32, kind="ExternalOutput")
    out_lg = nc.dram_tensor("out_lg", [T, B, D], f32, kind="ExternalOutput")

    rg = [list(range(NC))]
    n_steps = T if steps is None else steps

    with tile.TileContext(nc) as tc:
        with (
            tc.tile_pool(name="wpool", bufs=1) as wp,
            tc.tile_pool(name="state", bufs=1) as st,
            tc.tile_pool(name="xio", bufs=4) as xio,
            tc.tile_pool(name="tmp", bufs=2) as tp,
            tc.tile_pool(name="psg", bufs=1, space="PSUM") as psg,
            tc.tile_pool(name="psh", bufs=1, space="PSUM") as psh,
            tc.tile_pool(name="pst", bufs=1, space="PSUM") as pst,
            tc.tile_pool(name="dio", bufs=2, space="DRAM") as dio,
        ):
            # ---- load weights into SBUF (persistent) ----
            w = {}
            for name, dt_ in list(di.items()):
                if name in ("xT", "eps_f"):
                    continue
                shape = list(di[name].shape)
                t = wp.tile(shape, di[name].dtype, tag=f"w_{name}", name=f"w_{name}")
                nc.sync.dma_start(t[:], di[name][:])
                w[name] = t

            # ---- persistent state ----
            c_st = {}
            g_h = {}
            for n in lay_names:
                c_st[n] = st.tile([B, SL], f32, tag=f"c_{n}", name=f"c_{n}")
                nc.vector.memset(c_st[n][:], 0.0)
                g_h[n] = st.tile([128, NC * B], bf16, tag=f"gh_{n}", name=f"gh_{n}")
                nc.vector.memset(g_h[n][:], 0.0)
            # sigT holds sigmoid(logits(t-1))^T; sigmoid(0) = 0.5 at t=0
            sigT = st.tile([128, 2 * B], bf16, tag="sigT", name="sigT")
            nc.vector.memset(sigT[:], 0.5)

            ident64 = w["ident"][0:64, 0:64]

            def transpose_to(dst_ap, src_ap):
                """src [64,<=128] sbuf -> dst [<=128,64] sbuf (via PE + copy)."""
                ptile = pst.tile([128, B], bf16, tag="pt", name="pt")
                nc.tensor.transpose(ptile[:], src_ap, ident64)
                nc.vector.tensor_copy(dst_ap, ptile[:])

            def emit_bias_rec(name, psum):
                """bias + recurrent terms into psum (off critical path)."""
                whh, b = w[f"whh_{name}"], w[f"b_{name}"]
                nc.tensor.matmul(
                    psum[:, :], w["ones"][0:1, 0:B], b[0:1, :], start=True, stop=False
                )
                ghl = g_h[name]
                for cix in range(8):
                    nc.tensor.matmul(
                        psum[:, :],
                        ghl[:, 64 * cix : 64 * cix + 64],
                        whh[:, cix * G : (cix + 1) * G],
                        start=False,
                        stop=False,
                    )

            def emit_input(name, psum, in_stat, wkey=None):
                wih = w[wkey or f"wih_{name}"]
                n_in = len(in_stat)
                for cix, stat in enumerate(in_stat):
                    nc.tensor.matmul(
                        psum[:, :],
                        stat,
                        wih[:, cix * G : (cix + 1) * G],
                        start=False,
                        stop=False,
                    )

            def emit_stop(name, psum, in_stat, wkey=None):
                """last input chunk with stop=True"""
                wih = w[wkey or f"wih_{name}"]
                nc.tensor.matmul(
                    psum[:, :], in_stat[0], wih[:, in_stat[1] * G : (in_stat[1] + 1) * G],
                    start=False, stop=True,
                )

            def emit_cell(name, psum):
                """gates psum -> new h (bf16 [64,128]) ; updates c state."""
                tanh_g = tp.tile([B, 128], f32, tag="tanh_g", name="tanh_g")
                nc.scalar.activation(tanh_g[:], psum[:, 256:384], AF.Tanh)
                sig_if = tp.tile([B, 256], f32, tag="sig_if", name="sig_if")
                nc.scalar.activation(sig_if[:], psum[:, 0:256], AF.Sigmoid)
                sig_o = tp.tile([B, 128], f32, tag="sig_o", name="sig_o")
                nc.scalar.activation(sig_o[:], psum[:, 384:512], AF.Sigmoid)
                t2 = tp.tile([B, 128], f32, tag="t2", name="t2")
                nc.vector.tensor_mul(t2[:], sig_if[:, 128:256], c_st[name][:])
                t1 = tp.tile([B, 128], f32, tag="t1", name="t1")
                nc.vector.tensor_mul(t1[:], sig_if[:, 0:128], tanh_g[:])
                nc.vector.tensor_add(c_st[name][:], t1[:], t2[:])
                tanh_c = tp.tile([B, 128], f32, tag="tanh_c", name="tanh_c")
                nc.scalar.activation(tanh_c[:], c_st[name][:], AF.Tanh)
                h_bf = tp.tile([B, 128], bf16, tag="h_bf", name="h_bf")
                nc.vector.tensor_mul(h_bf[:], sig_o[:], tanh_c[:])
                return h_bf

            def emit_gather(name, h_bf):
                send = tp.tile([128, B], bf16, tag="send", name="send")
                transpose_to(send[:], h_bf[:])
                if gather_mode == "none":
                    for s in range(NC):
                        nc.vector.tensor_copy(g_h[name][:, s * B : (s + 1) * B], send[:])
                    return
                ib = dio.tile([128, B], bf16, tag="ib", name="ib")
                nc.sync.dma_start(ib[:], send[:])
                ob = dio.tile([NC, 128, B], bf16, tag="ob", name="ob")
                nc.gpsimd.collective_compute(
                    "AllGather",
                    mybir.AluOpType.bypass,
                    replica_groups=rg,
                    ins=[ib.opt()],
                    outs=[ob.opt()],
                )
                engs = [nc.sync, nc.scalar]
                for s in range(NC):
                    engs[s % 2].dma_start(g_h[name][:, s * B : (s + 1) * B], ob[s])

            def emit_head(wname, bname, stat_buf, psum, ncols):
                emit_head2(wname, bname, stat_buf, psum[:, :], ncols)

            def emit_head2(wname, bname, stat_buf, psum_ap, ncols):
                nc.tensor.matmul(
                    psum_ap, w["ones"][0:1, 0:B], w[bname][0:1, :],
                    start=True, stop=False,
                )
                for cix in range(8):
                    nc.tensor.matmul(
                        psum_ap,
                        stat_buf[:, 64 * cix : 64 * cix + 64],
                        w[wname][:, cix * ncols : (cix + 1) * ncols],
                        start=False,
                        stop=(cix == 7),
                    )

            # ================= time loop =================
            for t in range(n_steps):
                xT_t = xio.tile([128, 2 * B], bf16, tag="xT", name="xT")
                nc.sync.dma_start(xT_t[:], di["xT"][t])
                eps_t = xio.tile([B, D], f32, tag="eps", name="eps")
                nc.sync.dma_start(eps_t[:], di["eps_f"][t])

                # ---- bias + recurrent for all layers, plus e0's x-terms:
                # everything that does not depend on this step's gathers.
                psums = {}
                for name in lay_names:
                    psums[name] = psg.tile([B, G], f32, tag=f"ps_{name}", name=f"ps_{name}")
                    emit_bias_rec(name, psums[name])
                emit_input("e0", psums["e0"], [xT_t[:, 0:64], xT_t[:, 64:128]])

                # ---- encoder ----
                # e0: only the sigT terms are chain-dependent
                emit_input("e0", psums["e0"], [sigT[:, 0:64]], wkey="wn2_e0")
                emit_stop("e0", psums["e0"], (sigT[:, 64:128], 1), wkey="wn2_e0")
                h_bf = emit_cell("e0", psums["e0"])
                emit_gather("e0", h_bf)

                for name, prev in (("e1", "e0"), ("e2", "e1")):
                    pg = g_h[prev]
                    emit_input(name, psums[name],
                               [pg[:, 64 * cx : 64 * cx + 64] for cx in range(7)])
                    emit_stop(name, psums[name], (pg[:, 448:512], 7))
                    h_bf = emit_cell(name, psums[name])
                    emit_gather(name, h_bf)

                # ---- fused mu/logvar head + z ----
                pm = psh.tile([B, 2 * D], f32, tag="psh_muv", name="psh_muv")
                emit_head("w_muv", "b_muv", g_h["e2"], pm, 2 * D)
                muv_sb = tp.tile([B, 2 * D], f32, tag="muv_sb", name="muv_sb")
                nc.vector.tensor_copy(muv_sb[:], pm[:, :])
                nc.sync.dma_start(out_mu[t], muv_sb[:, 0:D])
                nc.sync.dma_start(out_lv[t], muv_sb[:, D : 2 * D])

                std_t = tp.tile([B, D], f32, tag="std", name="std")
                nc.scalar.activation(std_t[:], muv_sb[:, D : 2 * D], AF.Exp, scale=0.5)
                tz = tp.tile([B, D], f32, tag="tz", name="tz")
                nc.vector.tensor_mul(tz[:], eps_t[:], std_t[:])
                z_sb = tp.tile([B, D], f32, tag="z_sb", name="z_sb")
                nc.vector.tensor_add(z_sb[:], muv_sb[:, 0:D], tz[:])
                nc.sync.dma_start(out_z[t], z_sb[:])
                z_bf = tp.tile([B, D], bf16, tag="z_bf", name="z_bf")
                nc.vector.tensor_copy(z_bf[:], z_sb[:])
                zT = tp.tile([128, 2 * B], bf16, tag="zT", name="zT")
                for cix in range(2):
                    transpose_to(
                        zT[:, cix * B : (cix + 1) * B],
                        z_bf[:, cix * 128 : (cix + 1) * 128],
                    )

                # ---- decoder ----
                emit_input("d0", psums["d0"], [zT[:, 0:64]])
                emit_stop("d0", psums["d0"], (zT[:, 64:128], 1))
                h_bf = emit_cell("d0", psums["d0"])
                emit_gather("d0", h_bf)

                for name, prev in (("d1", "d0"), ("d2", "d1")):
                    pg = g_h[prev]
                    emit_input(name, psums[name],
                               [pg[:, 64 * cx : 64 * cx + 64] for cx in range(7)])
                    emit_stop(name, psums[name], (pg[:, 448:512], 7))
                    h_bf = emit_cell(name, psums[name])
                    emit_gather(name, h_bf)

                # ---- output head ----
                po_full = psh.tile([B, 2 * D], f32, tag="psh_muv", name="psh_muv")
                po = po_full[:, 0:D]
                emit_head2("w_out", "b_out", g_h["d2"], po, D)
                lg_sb = tp.tile([B, D], f32, tag="lg_sb", name="lg_sb")
                nc.vector.tensor_copy(lg_sb[:], po)
                nc.sync.dma_start(out_lg[t], lg_sb[:])

                # ---- sigT for t+1 ----
                if t + 1 < n_steps:
                    sig_lg = tp.tile([B, D], bf16, tag="sig_lg", name="sig_lg")
                    nc.scalar.activation(sig_lg[:], po, AF.Sigmoid)
                    for cix in range(2):
                        transpose_to(
                            sigT[:, cix * B : (cix + 1) * B],
                            sig_lg[:, cix * 128 : (cix + 1) * 128],
                        )

    nc.compile()
    return nc


_CACHE = {}


def run(inputs, T=T_FULL, trace=False):
    if T not in _CACHE:
        _CACHE[T] = build(T)
    nc = _CACHE[T]
    in_maps = [prepare_core_inputs(inputs, k, T) for k in range(NC)]
    res = run_bass_kernel_spmd(nc, in_maps, core_ids=list(range(NC)), trace=trace)
    r = res.results[0]
    sig = lambda v: 1.0 / (1.0 + np.exp(-v))
    tb = lambda a: np.ascontiguousarray(np.swapaxes(a, 0, 1))
    logits = tb(r["out_lg"])
    return (
        sig(logits).astype(np.float32),
        logits,
        tb(r["out_mu"]),
        tb(r["out_lv"]),
        tb(r["out_z"]),
    )


def kernel(**inputs):
    inputs = {k: np.asarray(v) for k, v in inputs.items()}
    return run(inputs, T=T_FULL)
